# revision 9
# baseline (speedup 1.0000x reference)
"""DetectHead (three 1x1-conv heads fused) on 8 Trainium2 NeuronCores.

Math: out[b,h,w,:] = concat(cls, box, dir) = W_all @ x[b,:,h,w] + bias_all
with W_all = concat(cls_w, box_w, dir_w) in R^{72x1024}.

Sharding: 8 shards = (batch, H-half). Each core processes a contiguous
(1024, 100*176=17600) slice of x and produces (17600, 72) of the
channels-last output.

The kernel is HBM-read-bound, so the main lever is input bytes. x is
quantized host-side to fp8 e4m3 (relative rounding err 2^-4; the matmul
contracts 1024 of them so the output rel err lands at ~1.1e-2, inside the
2e-2 gate).  Weights are also e4m3 — required for the PE's DoubleRow perf
mode (2 K-rows/cycle, both operands must be fp8e4/e5) — but split per
output channel o into  w = s_o * (hi + lo)  with hi = e4m3(w/s_o),
lo = e4m3(w/s_o - hi), s_o = max|w_o|/240.  The per-channel scale keeps
box_w (~1e-3) clear of e4m3's 2^-9 subnormal floor and the hi+lo pair
kills the weight quantization error (~1e-3 residual).

Per-tile pipeline (n=512 pixels):
  - 8 accumulating DoubleRow matmuls (4 k-pair chunks x {hi,lo},
    0.5 cycles/row; weight k-pitch padded to 80 for the ldweights
    step%16==0 ISA rule) -> PSUM [72,512] fp32
  - ACT engine activation(Copy, scale=s_o) applies the dequant scale
    during the PSUM->SBUF copy, writing fp16 rows 0..71 of a [73,512]
    staging tile whose row 72 is a persistent 1.0 (primed once)
  - 4 PE transposes against D[73,72] = [I; bias_row] — the ones row adds
    the bias for free — writing fp16 [128,72] tiles to PSUM
  - DVE copies the fp16 [128, nj*72] PSUM tile to SBUF (~150 ns),
    one DMA on the ACT ring writes the tile with dev_pixel = p*4 + j
    (576 B contiguous per partition -> full DMA efficiency); the host
    de-interleaves when gathering.

Engine budget per core: DMA ~57 us (17.6 MB fp8 in + 2.5 MB fp16 out at
360 GB/s) is the roofline; PE ~34-50 us (p-state dependent), ACT ~15 us,
DVE ~5 us all hide under it.
"""

import numpy as np
from contextlib import ExitStack

import ml_dtypes

import concourse.bass as bass
import concourse.tile as tile
from concourse import bacc, mybir
from concourse.bass_utils import run_bass_kernel_spmd

B, C, H, W = 4, 1024, 200, 176
HH = H // 2            # 100 rows of H per shard
PIX = HH * W           # 17600 pixels per shard
NCORES = 8
KCH = C // 128         # 8 channel chunks
O = 72                 # 18 cls + 42 box + 12 dir output channels
TILE_N = 512
FULL_TILES = PIX // TILE_N          # 34
TAIL = PIX - FULL_TILES * TILE_N    # 192

F32 = mybir.dt.float32
F16 = mybir.dt.float16
BF16 = mybir.dt.bfloat16
F8E4 = mybir.dt.float8e4
WPAD = 80  # ktile stride for fp8 weights: DoubleRow ldweights needs step%16==0

E4M3 = ml_dtypes.float8_e4m3
WSCALE_TARGET = 240.0  # normalize max|w_o| to this inside e4m3's range

_compiled = {}


def _build_program(repeat=1, group=4096, xbufs=3, mode="fp8hi"):
    nc = bacc.Bacc(
        "TRN2", target_bir_lowering=False, debug=False, num_devices=NCORES
    )
    if mode == "fp8dr":
        xdt, wdt, n_wk, wpitch = F8E4, F8E4, 2 * KCH, WPAD
    elif mode == "fp8hi":
        xdt, wdt, n_wk, wpitch = F8E4, F8E4, KCH, WPAD
    elif mode == "bf16":
        xdt, wdt, n_wk, wpitch = BF16, BF16, KCH, O
    else:
        raise ValueError(mode)

    xs = nc.dram_tensor("xs", [C, PIX], xdt, kind="ExternalInput").ap()
    wt = nc.dram_tensor("wt", [128, n_wk, wpitch], wdt, kind="ExternalInput").ap()
    svec = nc.dram_tensor("svec", [O, 1], F32, kind="ExternalInput").ap()
    bvec = nc.dram_tensor("bvec", [O, 1], F32, kind="ExternalInput").ap()
    dmat = nc.dram_tensor("dmat", [O, O], F16, kind="ExternalInput").ap()
    out = nc.dram_tensor("out", [PIX, O], F16, kind="ExternalOutput").ap()

    # [c, pix] viewed as [p, k, pix] with c = k*128 + p
    xs_v = xs.rearrange("(k p) n -> p k n", k=KCH)

    with tile.TileContext(nc) as tc, ExitStack() as ctx:
        cpool = ctx.enter_context(tc.tile_pool(name="consts", bufs=1))
        xpool = ctx.enter_context(tc.tile_pool(name="xin", bufs=xbufs))
        opool = ctx.enter_context(tc.tile_pool(name="outsb", bufs=3))
        mpool = ctx.enter_context(tc.tile_pool(name="pmm", bufs=2, space="PSUM"))
        tpool = ctx.enter_context(tc.tile_pool(name="ptr", bufs=2, space="PSUM"))

        w_sb = cpool.tile([128, n_wk, wpitch], wdt)
        nc.sync.dma_start(out=w_sb[:, :, :], in_=wt[:, :, :])
        s_sb = cpool.tile([O, 1], F32)
        nc.sync.dma_start(out=s_sb[:, :], in_=svec[:, :])
        b_sb = cpool.tile([O, 1], F32)
        nc.sync.dma_start(out=b_sb[:, :], in_=bvec[:, :])
        d_sb = cpool.tile([O, O], F16)
        nc.sync.dma_start(out=d_sb[:, :], in_=dmat[:, :])
        spool = ctx.enter_context(tc.tile_pool(name="stage", bufs=3))

        def emit_mms(xbuf, off, pix0, n):
            # accumulating matmuls for one n<=512 pixel tile -> PSUM [72, n]
            pmm = mpool.tile([O, n], F32, tag="pmm")
            if mode == "fp8dr":
                # 4 k-pair chunks x {hi, lo} accumulating DoubleRow matmuls
                for h in range(2):
                    for j in range(KCH // 2):
                        nc.tensor.matmul(
                            pmm[:, :],
                            w_sb[:, h * KCH + 2 * j : h * KCH + 2 * j + 2, :O],
                            xbuf[:, 2 * j : 2 * j + 2, off : off + n],
                            start=(h == 0 and j == 0),
                            stop=(h == 1 and j == KCH // 2 - 1),
                            perf_mode=mybir.MatmulPerfMode.DoubleRow,
                        )
            elif mode == "fp8hi":
                # 4 k-pair chunks, hi-precision weights only (single pass)
                for j in range(KCH // 2):
                    nc.tensor.matmul(
                        pmm[:, :],
                        w_sb[:, 2 * j : 2 * j + 2, :O],
                        xbuf[:, 2 * j : 2 * j + 2, off : off + n],
                        start=(j == 0),
                        stop=(j == KCH // 2 - 1),
                        perf_mode=mybir.MatmulPerfMode.DoubleRow,
                    )
            else:
                for k in range(KCH):
                    nc.tensor.matmul(
                        pmm[:, :],
                        w_sb[:, k, :O],
                        xbuf[:, k, off : off + n],
                        start=(k == 0),
                        stop=(k == KCH - 1),
                    )
            return pmm, pix0, n

        def emit_rest(pending):
            # scale-copy, transposes, SBUF stage, output DMA for a tile
            pmm, pix0, n = pending
            njs = [128] * (n // 128)
            if n % 128:
                njs.append(n % 128)
            nj = len(njs)

            # ACT: PSUM -> SBUF fp16, dequant scale and bias fused
            # (pre-transpose the output channel o is the partition dim, so
            # both are per-partition [72,1] vectors)
            s1 = spool.tile([O, TILE_N], F16, tag="s1")
            nc.scalar.activation(
                s1[:, :n], pmm[:, :],
                mybir.ActivationFunctionType.Identity,
                bias=b_sb[:, :],
                scale=s_sb[:, :],
            )

            # transpose via REGULAR matmul against an identity rhs:
            # out[pj,72] = s1_j.T @ I.  Regular matmuls warm-clock (2.4 GHz)
            # and FWL-load the fp16 stationary operand; transpose-mode
            # (nc.tensor.transpose) stays cold at 1.2 GHz and costs ~275 ns
            # per block vs ~100 ns for this form.  Output is fp32 in PSUM;
            # the DVE copy casts to fp16 on the way to SBUF.
            pt = tpool.tile([128, nj * O], F32, tag="pt")
            for j, pj in enumerate(njs):
                nc.tensor.matmul(
                    pt[:pj, j * O : (j + 1) * O],
                    s1[:, j * 128 : j * 128 + pj],
                    d_sb[:, :],
                    start=True,
                    stop=True,
                )

            ot = opool.tile([128, nj * O], F16, tag="ot")
            nc.vector.tensor_copy(ot[: max(njs), : nj * O], pt[: max(njs), : nj * O])
            if n % 128 == 0:
                # dev layout: dev_pixel = pix0 + p*nj + j  (576 B contiguous
                # per partition -> no sub-512B DMA penalty); host unpermutes.
                nc.scalar.dma_start(
                    out=out[pix0 : pix0 + n, :].rearrange(
                        "(p j) o -> p j o", p=128
                    ),
                    in_=ot[:, :].rearrange("p (j o) -> p j o", j=nj),
                )
            else:
                for j, pj in enumerate(njs):
                    nc.scalar.dma_start(
                        out=out[pix0 + j * 128 : pix0 + j * 128 + pj, :],
                        in_=ot[:pj, j * O : (j + 1) * O],
                    )

        # Software-pipelined emission: tile t+1's matmuls are issued BEFORE
        # tile t's transposes so the in-order PE queue never stalls waiting
        # for the ACT scale-copy — PE stays continuously busy and ramps to
        # its full 2.4 GHz p-state instead of oscillating at half speed.
        #
        # The group schedule is tapered: big groups for the bulk (DMA
        # efficiency), small final group so the compute tail after the last
        # input byte lands is one tile, not a whole group.
        GROUP = group  # pixels per input DMA (4096 -> 4 MB at fp8)
        schedule = []
        left = PIX
        while left > 0:
            gn = min(GROUP, left)
            schedule.append(gn)
            left -= gn
        if schedule[-1] > 2 * TILE_N and len(schedule) >= 1:
            last = schedule.pop()
            schedule += [last - TILE_N, TILE_N]
        pending = None
        for _rep in range(repeat):
            g0 = 0
            for gn in schedule:
                xbuf = xpool.tile([128, KCH, gn], xdt, tag="xbuf")
                nc.sync.dma_start(
                    out=xbuf[:, :, :], in_=xs_v[:, :, g0 : g0 + gn]
                )
                off = 0
                while off < gn:
                    m = min(TILE_N, gn - off)
                    nxt = emit_mms(xbuf, off, g0 + off, m)
                    if pending is not None:
                        emit_rest(pending)
                    pending = nxt
                    off += m
                g0 += gn
        if pending is not None:
            emit_rest(pending)

    nc.compile()
    return nc


def _get_program(repeat=1, group=4096, xbufs=3, mode="fp8hi"):
    key = (repeat, group, xbufs, mode)
    if key not in _compiled:
        _compiled[key] = _build_program(repeat, group, xbufs, mode)
    return _compiled[key]


def _make_in_maps(x, cls_w, cls_b, box_w, box_b, dir_w, dir_b, mode="fp8hi"):
    w_all = np.concatenate(
        [np.asarray(cls_w), np.asarray(box_w), np.asarray(dir_w)], axis=0
    ).astype(np.float32)  # (72, 1024)
    bias_all = np.concatenate(
        [np.asarray(cls_b), np.asarray(box_b), np.asarray(dir_b)]
    ).astype(np.float32)  # (72,)

    if mode == "fp8dr":
        s = np.abs(w_all).max(axis=1) / WSCALE_TARGET  # (72,)
        wp = w_all / s[:, None]
        w_hi = wp.astype(E4M3)
        w_lo = (wp - w_hi.astype(np.float32)).astype(E4M3)
        # wt[p, h*KCH + k, o] = w_{hi,lo}[o, k*128 + p]
        whl = np.stack([w_hi, w_lo])  # (2, 72, 1024)
        wt = np.zeros((128, 2 * KCH, WPAD), dtype=E4M3)
        wt[:, :, :O] = whl.reshape(2, O, KCH, 128).transpose(3, 0, 2, 1).reshape(
            128, 2 * KCH, O
        )
        svec = s.reshape(O, 1).astype(np.float32)
        xq = np.asarray(x).astype(E4M3)
    elif mode == "fp8hi":
        # single-pass: per-channel-scaled e4m3 weights, no lo residual.
        # rel err ~1.59e-2 (vs 1.16e-2 with hi+lo), inside the 2e-2 gate.
        s = np.abs(w_all).max(axis=1) / WSCALE_TARGET  # (72,)
        w_hi = (w_all / s[:, None]).astype(E4M3)
        wt = np.zeros((128, KCH, WPAD), dtype=E4M3)
        wt[:, :, :O] = w_hi.reshape(O, KCH, 128).transpose(2, 1, 0)
        svec = s.reshape(O, 1).astype(np.float32)
        xq = np.asarray(x).astype(E4M3)
    else:
        wb = w_all.astype(ml_dtypes.bfloat16)
        wt = np.ascontiguousarray(
            wb.reshape(O, KCH, 128).transpose(2, 1, 0).reshape(128, KCH, O)
        )
        svec = np.ones((O, 1), dtype=np.float32)
        xq = np.asarray(x).astype(ml_dtypes.bfloat16)

    dmat = np.eye(O, dtype=np.float16)
    bvec = bias_all.reshape(O, 1).astype(np.float32)

    in_maps = []
    for i in range(NCORES):
        b, half = divmod(i, 2)
        xs = np.ascontiguousarray(
            xq[b, :, half * HH : (half + 1) * HH, :]
        ).reshape(C, PIX)
        in_maps.append(
            {"xs": xs, "wt": wt, "svec": svec, "bvec": bvec, "dmat": dmat}
        )
    return in_maps


def _chunks(group=4096):
    """(pix0, n) tile chunks in device-emission order, matching
    _build_program's tapered group schedule."""
    schedule = []
    left = PIX
    while left > 0:
        gn = min(group, left)
        schedule.append(gn)
        left -= gn
    if schedule[-1] > 2 * TILE_N:
        last = schedule.pop()
        schedule += [last - TILE_N, TILE_N]
    out, g0 = [], 0
    for gn in schedule:
        off = 0
        while off < gn:
            m = min(TILE_N, gn - off)
            out.append((g0 + off, m))
            off += m
        g0 += gn
    return out


def _gather(results, group=4096):
    out = np.empty((B, H, W, O), dtype=np.float32)
    for i in range(NCORES):
        b, half = divmod(i, 2)
        dev = results[i]["out"].astype(np.float32)  # (PIX, 72)
        flat = np.empty((PIX, O), dtype=np.float32)
        for pix0, n in _chunks(group):
            if n % 128 == 0:
                nj = n // 128
                # interleaved tile: dev_pixel = pix0 + p*nj + j
                flat[pix0 : pix0 + n] = (
                    dev[pix0 : pix0 + n]
                    .reshape(128, nj, O)
                    .transpose(1, 0, 2)
                    .reshape(n, O)
                )
            else:
                flat[pix0 : pix0 + n] = dev[pix0 : pix0 + n]
        out[b, half * HH : (half + 1) * HH] = flat.reshape(HH, W, O)
    return out


def kernel(x, cls_w, cls_b, box_w, box_b, dir_w, dir_b):
    nc = _get_program()
    in_maps = _make_in_maps(x, cls_w, cls_b, box_w, box_b, dir_w, dir_b)
    res = run_bass_kernel_spmd(nc, in_maps, list(range(NCORES)))
    return _gather(res.results)



# revision 14
# speedup vs baseline: 1.1549x; 1.1549x over previous
"""DetectHead (three 1x1-conv heads fused) on 8 Trainium2 NeuronCores.

Math: out[b,h,w,:] = concat(cls, box, dir) = W_all @ x[b,:,h,w] + bias_all
with W_all = concat(cls_w, box_w, dir_w) in R^{72x1024}.

Sharding: 8 shards = (batch, H-half). Each core processes a contiguous
(1024, 100*176=17600) slice of x and produces (17600, 72) of the
channels-last output.

The kernel is HBM-read-bound, so the main lever is input bytes. x is
quantized host-side to fp8 e4m3 (relative rounding err 2^-4; the matmul
contracts 1024 of them so the output rel err lands at ~1.1e-2, inside the
2e-2 gate).  Weights are also e4m3 — required for the PE's DoubleRow perf
mode (2 K-rows/cycle, both operands must be fp8e4/e5) — but split per
output channel o into  w = s_o * (hi + lo)  with hi = e4m3(w/s_o),
lo = e4m3(w/s_o - hi), s_o = max|w_o|/240.  The per-channel scale keeps
box_w (~1e-3) clear of e4m3's 2^-9 subnormal floor and the hi+lo pair
kills the weight quantization error (~1e-3 residual).

Per-tile pipeline (n=512 pixels):
  - 8 accumulating DoubleRow matmuls (4 k-pair chunks x {hi,lo},
    0.5 cycles/row; weight k-pitch padded to 80 for the ldweights
    step%16==0 ISA rule) -> PSUM [72,512] fp32
  - ACT engine activation(Copy, scale=s_o) applies the dequant scale
    during the PSUM->SBUF copy, writing fp16 rows 0..71 of a [73,512]
    staging tile whose row 72 is a persistent 1.0 (primed once)
  - 4 PE transposes against D[73,72] = [I; bias_row] — the ones row adds
    the bias for free — writing fp16 [128,72] tiles to PSUM
  - DVE copies the fp16 [128, nj*72] PSUM tile to SBUF (~150 ns),
    one DMA on the ACT ring writes the tile with dev_pixel = p*4 + j
    (576 B contiguous per partition -> full DMA efficiency); the host
    de-interleaves when gathering.

Engine budget per core: DMA ~57 us (17.6 MB fp8 in + 2.5 MB fp16 out at
360 GB/s) is the roofline; PE ~34-50 us (p-state dependent), ACT ~15 us,
DVE ~5 us all hide under it.
"""

import numpy as np
from contextlib import ExitStack

import ml_dtypes

import concourse.bass as bass
import concourse.tile as tile
from concourse import bacc, mybir
from concourse.bass_utils import run_bass_kernel_spmd

B, C, H, W = 4, 1024, 200, 176
HH = H // 2            # 100 rows of H per shard
PIX = HH * W           # 17600 pixels per shard
NCORES = 8
KCH = C // 128         # 8 channel chunks
O = 72                 # 18 cls + 42 box + 12 dir output channels
TILE_N = 512
FULL_TILES = PIX // TILE_N          # 34
TAIL = PIX - FULL_TILES * TILE_N    # 192

F32 = mybir.dt.float32
F16 = mybir.dt.float16
BF16 = mybir.dt.bfloat16
F8E4 = mybir.dt.float8e4
WPAD = 80  # ktile stride for fp8 weights: DoubleRow ldweights needs step%16==0

E4M3 = ml_dtypes.float8_e4m3
WSCALE_TARGET = 240.0  # normalize max|w_o| to this inside e4m3's range

_compiled = {}


def _build_program(repeat=1, group=4096, xbufs=3, mode="fp8hi",
                   trmode="pemm", oring="scalar", odefer=1):
    nc = bacc.Bacc(
        "TRN2", target_bir_lowering=False, debug=False, num_devices=NCORES
    )
    if mode == "fp8dr":
        xdt, wdt, n_wk, wpitch = F8E4, F8E4, 2 * KCH, WPAD
    elif mode == "fp8hi":
        xdt, wdt, n_wk, wpitch = F8E4, F8E4, KCH, WPAD
    elif mode == "bf16":
        xdt, wdt, n_wk, wpitch = BF16, BF16, KCH, O
    else:
        raise ValueError(mode)

    xs = nc.dram_tensor("xs", [C, PIX], xdt, kind="ExternalInput").ap()
    wt = nc.dram_tensor("wt", [128, n_wk, wpitch], wdt, kind="ExternalInput").ap()
    svec = nc.dram_tensor("svec", [O, 1], F32, kind="ExternalInput").ap()
    bvec = nc.dram_tensor("bvec", [O, 1], F32, kind="ExternalInput").ap()
    dmat = nc.dram_tensor("dmat", [O, O], F16, kind="ExternalInput").ap()
    out = nc.dram_tensor("out", [PIX, O], F16, kind="ExternalOutput").ap()

    # [c, pix] viewed as [p, k, pix] with c = k*128 + p
    xs_v = xs.rearrange("(k p) n -> p k n", k=KCH)

    with tile.TileContext(nc) as tc, ExitStack() as ctx:
        cpool = ctx.enter_context(tc.tile_pool(name="consts", bufs=1))
        xpool = ctx.enter_context(tc.tile_pool(name="xin", bufs=xbufs))
        opool = ctx.enter_context(
            tc.tile_pool(name="outsb", bufs=3 + odefer)
        )
        mpool = ctx.enter_context(tc.tile_pool(name="pmm", bufs=2, space="PSUM"))
        tpool = ctx.enter_context(tc.tile_pool(name="ptr", bufs=2, space="PSUM"))
        dma_eng = {"scalar": nc.scalar, "sync": nc.sync,
                   "gpsimd": nc.gpsimd}[oring]

        w_sb = cpool.tile([128, n_wk, wpitch], wdt)
        nc.sync.dma_start(out=w_sb[:, :, :], in_=wt[:, :, :])
        s_sb = cpool.tile([O, 1], F32)
        nc.sync.dma_start(out=s_sb[:, :], in_=svec[:, :])
        b_sb = cpool.tile([O, 1], F32)
        nc.sync.dma_start(out=b_sb[:, :], in_=bvec[:, :])
        d_sb = cpool.tile([O, O], F16)
        nc.sync.dma_start(out=d_sb[:, :], in_=dmat[:, :])
        spool = ctx.enter_context(tc.tile_pool(name="stage", bufs=3))

        def emit_mms(xbuf, off, pix0, n):
            # accumulating matmuls for one n<=512 pixel tile -> PSUM [72, n]
            pmm = mpool.tile([O, n], F32, tag="pmm")
            if mode == "fp8dr":
                # 4 k-pair chunks x {hi, lo} accumulating DoubleRow matmuls
                for h in range(2):
                    for j in range(KCH // 2):
                        nc.tensor.matmul(
                            pmm[:, :],
                            w_sb[:, h * KCH + 2 * j : h * KCH + 2 * j + 2, :O],
                            xbuf[:, 2 * j : 2 * j + 2, off : off + n],
                            start=(h == 0 and j == 0),
                            stop=(h == 1 and j == KCH // 2 - 1),
                            perf_mode=mybir.MatmulPerfMode.DoubleRow,
                        )
            elif mode == "fp8hi":
                # 4 k-pair chunks, hi-precision weights only (single pass)
                for j in range(KCH // 2):
                    nc.tensor.matmul(
                        pmm[:, :],
                        w_sb[:, 2 * j : 2 * j + 2, :O],
                        xbuf[:, 2 * j : 2 * j + 2, off : off + n],
                        start=(j == 0),
                        stop=(j == KCH // 2 - 1),
                        perf_mode=mybir.MatmulPerfMode.DoubleRow,
                    )
            else:
                for k in range(KCH):
                    nc.tensor.matmul(
                        pmm[:, :],
                        w_sb[:, k, :O],
                        xbuf[:, k, off : off + n],
                        start=(k == 0),
                        stop=(k == KCH - 1),
                    )
            return pmm, pix0, n

        def emit_rest(pending):
            # scale-copy, transposes, SBUF stage, output DMA for a tile
            pmm, pix0, n = pending
            njs = [128] * (n // 128)
            if n % 128:
                njs.append(n % 128)
            nj = len(njs)

            # ACT: PSUM -> SBUF fp16, dequant scale and bias fused
            # (pre-transpose the output channel o is the partition dim, so
            # both are per-partition [72,1] vectors)
            s1 = spool.tile([O, TILE_N], F16, tag="s1")
            nc.scalar.activation(
                s1[:, :n], pmm[:, :],
                mybir.ActivationFunctionType.Identity,
                bias=b_sb[:, :],
                scale=s_sb[:, :],
            )

            # transpose via REGULAR matmul against an identity rhs:
            # out[pj,72] = s1_j.T @ I.  Regular matmuls warm-clock (2.4 GHz)
            # and FWL-load the fp16 stationary operand; transpose-mode
            # (nc.tensor.transpose) stays cold at 1.2 GHz.  Output is fp32
            # in PSUM; the DVE copy casts to fp16 on the way to SBUF.
            if trmode == "pemm":
                pt = tpool.tile([128, nj * O], F32, tag="pt")
                for j, pj in enumerate(njs):
                    nc.tensor.matmul(
                        pt[:pj, j * O : (j + 1) * O],
                        s1[:, j * 128 : j * 128 + pj],
                        d_sb[:, :],
                        start=True,
                        stop=True,
                    )
            else:
                pt = tpool.tile([128, nj * O], F16, tag="pt")
                for j, pj in enumerate(njs):
                    nc.tensor.transpose(
                        pt[:pj, j * O : (j + 1) * O],
                        s1[:, j * 128 : j * 128 + pj],
                        d_sb[:, :],
                    )

            ot = opool.tile([128, nj * O], F16, tag="ot")
            nc.vector.tensor_copy(ot[: max(njs), : nj * O], pt[: max(njs), : nj * O])
            return ot, pix0, n, njs, nj

        def emit_dma(staged):
            # output DMA, deferred `odefer` tiles so the dma_start never
            # waits at its queue head for this tile's DVE copy (which would
            # block the next ACT behind it on a strict-FIFO engine queue)
            ot, pix0, n, njs, nj = staged
            if n % 128 == 0:
                # dev layout: dev_pixel = pix0 + p*nj + j  (576 B contiguous
                # per partition -> no sub-512B DMA penalty); host unpermutes.
                dma_eng.dma_start(
                    out=out[pix0 : pix0 + n, :].rearrange(
                        "(p j) o -> p j o", p=128
                    ),
                    in_=ot[:, :].rearrange("p (j o) -> p j o", j=nj),
                )
            else:
                for j, pj in enumerate(njs):
                    dma_eng.dma_start(
                        out=out[pix0 + j * 128 : pix0 + j * 128 + pj, :],
                        in_=ot[:pj, j * O : (j + 1) * O],
                    )

        # Software-pipelined emission: tile t+1's matmuls are issued BEFORE
        # tile t's transposes so the in-order PE queue never stalls waiting
        # for the ACT scale-copy — PE stays continuously busy and ramps to
        # its full 2.4 GHz p-state instead of oscillating at half speed.
        #
        # The group schedule is tapered: big groups for the bulk (DMA
        # efficiency), small final group so the compute tail after the last
        # input byte lands is one tile, not a whole group.
        GROUP = group  # pixels per input DMA (4096 -> 4 MB at fp8)
        schedule = []
        left = PIX
        while left > 0:
            gn = min(GROUP, left)
            schedule.append(gn)
            left -= gn
        if schedule[-1] > 2 * TILE_N and len(schedule) >= 1:
            last = schedule.pop()
            schedule += [last - TILE_N, TILE_N]
        pending = None
        dma_q = []
        for _rep in range(repeat):
            g0 = 0
            for gn in schedule:
                xbuf = xpool.tile([128, KCH, gn], xdt, tag="xbuf")
                nc.sync.dma_start(
                    out=xbuf[:, :, :], in_=xs_v[:, :, g0 : g0 + gn]
                )
                off = 0
                while off < gn:
                    m = min(TILE_N, gn - off)
                    nxt = emit_mms(xbuf, off, g0 + off, m)
                    if pending is not None:
                        dma_q.append(emit_rest(pending))
                        if len(dma_q) > odefer:
                            emit_dma(dma_q.pop(0))
                    pending = nxt
                    off += m
                g0 += gn
        if pending is not None:
            dma_q.append(emit_rest(pending))
        for staged in dma_q:
            emit_dma(staged)

    nc.compile()
    return nc


def _get_program(repeat=1, group=4096, xbufs=3, mode="fp8hi",
                 trmode="pemm", oring="scalar", odefer=1):
    key = (repeat, group, xbufs, mode, trmode, oring, odefer)
    if key not in _compiled:
        _compiled[key] = _build_program(
            repeat, group, xbufs, mode, trmode, oring, odefer
        )
    return _compiled[key]


def _make_in_maps(x, cls_w, cls_b, box_w, box_b, dir_w, dir_b, mode="fp8hi"):
    w_all = np.concatenate(
        [np.asarray(cls_w), np.asarray(box_w), np.asarray(dir_w)], axis=0
    ).astype(np.float32)  # (72, 1024)
    bias_all = np.concatenate(
        [np.asarray(cls_b), np.asarray(box_b), np.asarray(dir_b)]
    ).astype(np.float32)  # (72,)

    if mode == "fp8dr":
        s = np.abs(w_all).max(axis=1) / WSCALE_TARGET  # (72,)
        wp = w_all / s[:, None]
        w_hi = wp.astype(E4M3)
        w_lo = (wp - w_hi.astype(np.float32)).astype(E4M3)
        # wt[p, h*KCH + k, o] = w_{hi,lo}[o, k*128 + p]
        whl = np.stack([w_hi, w_lo])  # (2, 72, 1024)
        wt = np.zeros((128, 2 * KCH, WPAD), dtype=E4M3)
        wt[:, :, :O] = whl.reshape(2, O, KCH, 128).transpose(3, 0, 2, 1).reshape(
            128, 2 * KCH, O
        )
        svec = s.reshape(O, 1).astype(np.float32)
        xq = np.asarray(x).astype(E4M3)
    elif mode == "fp8hi":
        # single-pass: per-channel-scaled e4m3 weights, no lo residual.
        # rel err ~1.59e-2 (vs 1.16e-2 with hi+lo), inside the 2e-2 gate.
        s = np.abs(w_all).max(axis=1) / WSCALE_TARGET  # (72,)
        w_hi = (w_all / s[:, None]).astype(E4M3)
        wt = np.zeros((128, KCH, WPAD), dtype=E4M3)
        wt[:, :, :O] = w_hi.reshape(O, KCH, 128).transpose(2, 1, 0)
        svec = s.reshape(O, 1).astype(np.float32)
        xq = np.asarray(x).astype(E4M3)
    else:
        wb = w_all.astype(ml_dtypes.bfloat16)
        wt = np.ascontiguousarray(
            wb.reshape(O, KCH, 128).transpose(2, 1, 0).reshape(128, KCH, O)
        )
        svec = np.ones((O, 1), dtype=np.float32)
        xq = np.asarray(x).astype(ml_dtypes.bfloat16)

    dmat = np.eye(O, dtype=np.float16)
    bvec = bias_all.reshape(O, 1).astype(np.float32)

    in_maps = []
    for i in range(NCORES):
        b, half = divmod(i, 2)
        xs = np.ascontiguousarray(
            xq[b, :, half * HH : (half + 1) * HH, :]
        ).reshape(C, PIX)
        in_maps.append(
            {"xs": xs, "wt": wt, "svec": svec, "bvec": bvec, "dmat": dmat}
        )
    return in_maps


def _chunks(group=4096):
    """(pix0, n) tile chunks in device-emission order, matching
    _build_program's tapered group schedule."""
    schedule = []
    left = PIX
    while left > 0:
        gn = min(group, left)
        schedule.append(gn)
        left -= gn
    if schedule[-1] > 2 * TILE_N:
        last = schedule.pop()
        schedule += [last - TILE_N, TILE_N]
    out, g0 = [], 0
    for gn in schedule:
        off = 0
        while off < gn:
            m = min(TILE_N, gn - off)
            out.append((g0 + off, m))
            off += m
        g0 += gn
    return out


def _gather(results, group=4096):
    out = np.empty((B, H, W, O), dtype=np.float32)
    for i in range(NCORES):
        b, half = divmod(i, 2)
        dev = results[i]["out"].astype(np.float32)  # (PIX, 72)
        flat = np.empty((PIX, O), dtype=np.float32)
        for pix0, n in _chunks(group):
            if n % 128 == 0:
                nj = n // 128
                # interleaved tile: dev_pixel = pix0 + p*nj + j
                flat[pix0 : pix0 + n] = (
                    dev[pix0 : pix0 + n]
                    .reshape(128, nj, O)
                    .transpose(1, 0, 2)
                    .reshape(n, O)
                )
            else:
                flat[pix0 : pix0 + n] = dev[pix0 : pix0 + n]
        out[b, half * HH : (half + 1) * HH] = flat.reshape(HH, W, O)
    return out


def kernel(x, cls_w, cls_b, box_w, box_b, dir_w, dir_b):
    nc = _get_program()
    in_maps = _make_in_maps(x, cls_w, cls_b, box_w, box_b, dir_w, dir_b)
    res = run_bass_kernel_spmd(nc, in_maps, list(range(NCORES)))
    return _gather(res.results)



# revision 17
# speedup vs baseline: 1.2056x; 1.0439x over previous
"""DetectHead (three 1x1-conv heads fused) on 8 Trainium2 NeuronCores.

Math: out[b,h,w,:] = concat(cls, box, dir) = W_all @ x[b,:,h,w] + bias_all
with W_all = concat(cls_w, box_w, dir_w) in R^{72x1024}.

Sharding: 8 shards = (batch, H-half). Each core processes a contiguous
(1024, 100*176=17600) slice of x and produces (17600, 72) of the
channels-last output.

The kernel is HBM-read-bound, so the main lever is input bytes. x is
quantized host-side to fp8 e4m3 (relative rounding err 2^-4; the matmul
contracts 1024 of them so the output rel err lands at ~1.1e-2, inside the
2e-2 gate).  Weights are also e4m3 — required for the PE's DoubleRow perf
mode (2 K-rows/cycle, both operands must be fp8e4/e5) — but split per
output channel o into  w = s_o * (hi + lo)  with hi = e4m3(w/s_o),
lo = e4m3(w/s_o - hi), s_o = max|w_o|/240.  The per-channel scale keeps
box_w (~1e-3) clear of e4m3's 2^-9 subnormal floor and the hi+lo pair
kills the weight quantization error (~1e-3 residual).

Per-tile pipeline (n=512 pixels):
  - 8 accumulating DoubleRow matmuls (4 k-pair chunks x {hi,lo},
    0.5 cycles/row; weight k-pitch padded to 80 for the ldweights
    step%16==0 ISA rule) -> PSUM [72,512] fp32
  - ACT engine activation(Copy, scale=s_o) applies the dequant scale
    during the PSUM->SBUF copy, writing fp16 rows 0..71 of a [73,512]
    staging tile whose row 72 is a persistent 1.0 (primed once)
  - 4 PE transposes against D[73,72] = [I; bias_row] — the ones row adds
    the bias for free — writing fp16 [128,72] tiles to PSUM
  - DVE copies the fp16 [128, nj*72] PSUM tile to SBUF (~150 ns),
    one DMA on the ACT ring writes the tile with dev_pixel = p*4 + j
    (576 B contiguous per partition -> full DMA efficiency); the host
    de-interleaves when gathering.

Engine budget per core: DMA ~57 us (17.6 MB fp8 in + 2.5 MB fp16 out at
360 GB/s) is the roofline; PE ~34-50 us (p-state dependent), ACT ~15 us,
DVE ~5 us all hide under it.
"""

import numpy as np
from contextlib import ExitStack

import ml_dtypes

import concourse.bass as bass
import concourse.tile as tile
from concourse import bacc, mybir
from concourse.bass_utils import run_bass_kernel_spmd

B, C, H, W = 4, 1024, 200, 176
HH = H // 2            # 100 rows of H per shard
PIX = HH * W           # 17600 pixels per shard
NCORES = 8
KCH = C // 128         # 8 channel chunks
O = 72                 # 18 cls + 42 box + 12 dir output channels
TILE_N = 512
FULL_TILES = PIX // TILE_N          # 34
TAIL = PIX - FULL_TILES * TILE_N    # 192

F32 = mybir.dt.float32
F16 = mybir.dt.float16
BF16 = mybir.dt.bfloat16
F8E4 = mybir.dt.float8e4
WPAD = 80  # ktile stride for fp8 weights: DoubleRow ldweights needs step%16==0

E4M3 = ml_dtypes.float8_e4m3
WSCALE_TARGET = 240.0  # normalize max|w_o| to this inside e4m3's range

_compiled = {}


def _build_program(repeat=1, group=4096, xbufs=3, mode="fp8hi",
                   trmode="pemm", oring="scalar", odefer=1, stages=7):
    nc = bacc.Bacc(
        "TRN2", target_bir_lowering=False, debug=False, num_devices=NCORES
    )
    if mode == "fp8dr":
        xdt, wdt, n_wk, wpitch = F8E4, F8E4, 2 * KCH, WPAD
    elif mode == "fp8hi":
        xdt, wdt, n_wk, wpitch = F8E4, F8E4, KCH, WPAD
    elif mode == "bf16":
        xdt, wdt, n_wk, wpitch = BF16, BF16, KCH, O
    else:
        raise ValueError(mode)

    xs = nc.dram_tensor("xs", [C, PIX], xdt, kind="ExternalInput").ap()
    wt = nc.dram_tensor("wt", [128, n_wk, wpitch], wdt, kind="ExternalInput").ap()
    svec = nc.dram_tensor("svec", [O, 1], F32, kind="ExternalInput").ap()
    bvec = nc.dram_tensor("bvec", [O, 1], F32, kind="ExternalInput").ap()
    dmat = nc.dram_tensor("dmat", [O, O], F16, kind="ExternalInput").ap()
    OP = 80 if trmode == "xbar" else O  # xbar: pad to %16 partitions
    out = nc.dram_tensor("out", [PIX, OP], F16, kind="ExternalOutput").ap()

    # [c, pix] viewed as [p, k, pix] with c = k*128 + p
    xs_v = xs.rearrange("(k p) n -> p k n", k=KCH)

    with tile.TileContext(nc) as tc, ExitStack() as ctx:
        cpool = ctx.enter_context(tc.tile_pool(name="consts", bufs=1))
        xpool = ctx.enter_context(tc.tile_pool(name="xin", bufs=xbufs))
        opool = ctx.enter_context(
            tc.tile_pool(name="outsb", bufs=3 + odefer)
        )
        mpool = ctx.enter_context(tc.tile_pool(name="pmm", bufs=2, space="PSUM"))
        tpool = ctx.enter_context(tc.tile_pool(name="ptr", bufs=2, space="PSUM"))
        dma_eng = {"scalar": nc.scalar, "sync": nc.sync,
                   "gpsimd": nc.gpsimd}[oring]

        w_sb = cpool.tile([128, n_wk, wpitch], wdt)
        nc.sync.dma_start(out=w_sb[:, :, :], in_=wt[:, :, :])
        s_sb = cpool.tile([O, 1], F32)
        nc.sync.dma_start(out=s_sb[:, :], in_=svec[:, :])
        b_sb = cpool.tile([O, 1], F32)
        nc.sync.dma_start(out=b_sb[:, :], in_=bvec[:, :])
        d_sb = cpool.tile([O, O], F16)
        nc.sync.dma_start(out=d_sb[:, :], in_=dmat[:, :])
        spool = ctx.enter_context(tc.tile_pool(name="stage", bufs=3))

        def emit_mms(xbuf, off, pix0, n):
            # accumulating matmuls for one n<=512 pixel tile -> PSUM [72, n]
            pmm = mpool.tile([O, n], F32, tag="pmm")
            if mode == "fp8dr":
                # 4 k-pair chunks x {hi, lo} accumulating DoubleRow matmuls
                for h in range(2):
                    for j in range(KCH // 2):
                        nc.tensor.matmul(
                            pmm[:, :],
                            w_sb[:, h * KCH + 2 * j : h * KCH + 2 * j + 2, :O],
                            xbuf[:, 2 * j : 2 * j + 2, off : off + n],
                            start=(h == 0 and j == 0),
                            stop=(h == 1 and j == KCH // 2 - 1),
                            perf_mode=mybir.MatmulPerfMode.DoubleRow,
                        )
            elif mode == "fp8hi":
                # 4 k-pair chunks, hi-precision weights only (single pass)
                for j in range(KCH // 2):
                    nc.tensor.matmul(
                        pmm[:, :],
                        w_sb[:, 2 * j : 2 * j + 2, :O],
                        xbuf[:, 2 * j : 2 * j + 2, off : off + n],
                        start=(j == 0),
                        stop=(j == KCH // 2 - 1),
                        perf_mode=mybir.MatmulPerfMode.DoubleRow,
                    )
            else:
                for k in range(KCH):
                    nc.tensor.matmul(
                        pmm[:, :],
                        w_sb[:, k, :O],
                        xbuf[:, k, off : off + n],
                        start=(k == 0),
                        stop=(k == KCH - 1),
                    )
            return pmm, pix0, n

        def emit_rest(pending):
            # scale-copy, transposes, SBUF stage, output DMA for a tile
            pmm, pix0, n = pending
            if not stages & 1:
                return None
            njs = [128] * (n // 128)
            if n % 128:
                njs.append(n % 128)
            nj = len(njs)

            # ACT: PSUM -> SBUF fp16, dequant scale and bias fused
            # (pre-transpose the output channel o is the partition dim, so
            # both are per-partition [72,1] vectors)
            s1 = spool.tile([OP if trmode == "xbar" else O, TILE_N], F16,
                            tag="s1")
            nc.scalar.activation(
                s1[:O, :n], pmm[:, :],
                mybir.ActivationFunctionType.Identity,
                bias=b_sb[:, :],
                scale=s_sb[:, :],
            )

            if not stages & 2:
                return None
            if trmode == "xbar":
                # X-bar DMA transpose, SBUF->SBUF on the HWDGE scalar ring:
                # [80,128] -> [128,80] per block; no PE/DVE involvement and
                # SBUF<->SBUF traffic does not eat the HBM budget.  Rows
                # 72..79 of s1 are garbage; they ship as cols 72..80 and the
                # host strips them.  The 64-px tail block (free dim %128 != 0)
                # falls back to the PE path below.
                ot = opool.tile([128, nj * OP], F16, tag="ot")
                pemm_js = []
                for j, pj in enumerate(njs):
                    if pj % 128 == 0:
                        nc.scalar.dma_start_transpose(
                            ot[:, j * OP : (j + 1) * OP],
                            s1[:, j * 128 : j * 128 + pj],
                        )
                    else:
                        pemm_js.append((j, pj))
                if pemm_js:
                    pt = tpool.tile([128, len(pemm_js) * O], F32, tag="pt")
                    for i, (j, pj) in enumerate(pemm_js):
                        nc.tensor.matmul(
                            pt[:pj, i * O : (i + 1) * O],
                            s1[:O, j * 128 : j * 128 + pj],
                            d_sb[:, :],
                            start=True,
                            stop=True,
                        )
                        nc.vector.tensor_copy(
                            ot[:pj, j * OP : j * OP + O],
                            pt[:pj, i * O : (i + 1) * O],
                        )
                return ot, pix0, n, njs, nj
            # transpose via REGULAR matmul against an identity rhs:
            # out[pj,72] = s1_j.T @ I.  Regular matmuls warm-clock (2.4 GHz)
            # and FWL-load the fp16 stationary operand; transpose-mode
            # (nc.tensor.transpose) stays cold at 1.2 GHz.  Output is fp32
            # in PSUM; the DVE copy casts to fp16 on the way to SBUF.
            if trmode == "pemm":
                pt = tpool.tile([128, nj * O], F32, tag="pt")
                for j, pj in enumerate(njs):
                    nc.tensor.matmul(
                        pt[:pj, j * O : (j + 1) * O],
                        s1[:, j * 128 : j * 128 + pj],
                        d_sb[:, :],
                        start=True,
                        stop=True,
                    )
            else:
                pt = tpool.tile([128, nj * O], F16, tag="pt")
                for j, pj in enumerate(njs):
                    nc.tensor.transpose(
                        pt[:pj, j * O : (j + 1) * O],
                        s1[:, j * 128 : j * 128 + pj],
                        d_sb[:, :],
                    )

            ot = opool.tile([128, nj * O], F16, tag="ot")
            nc.vector.tensor_copy(ot[: max(njs), : nj * O], pt[: max(njs), : nj * O])
            return ot, pix0, n, njs, nj

        def emit_dma(staged):
            # output DMA, deferred `odefer` tiles so the dma_start never
            # waits at its queue head for this tile's DVE copy (which would
            # block the next ACT behind it on a strict-FIFO engine queue)
            ot, pix0, n, njs, nj = staged
            if not stages & 4:
                return
            W_ = OP if trmode == "xbar" else O
            if n % 128 == 0:
                # dev layout: dev_pixel = pix0 + p*nj + j  (>=576 B contiguous
                # per partition -> no sub-512B DMA penalty); host unpermutes.
                dma_eng.dma_start(
                    out=out[pix0 : pix0 + n, :].rearrange(
                        "(p j) o -> p j o", p=128
                    ),
                    in_=ot[:, : nj * W_].rearrange("p (j o) -> p j o", j=nj),
                )
            else:
                for j, pj in enumerate(njs):
                    dma_eng.dma_start(
                        out=out[pix0 + j * 128 : pix0 + j * 128 + pj, :],
                        in_=ot[:pj, j * W_ : j * W_ + W_],
                    )

        # Software-pipelined emission: tile t+1's matmuls are issued BEFORE
        # tile t's transposes so the in-order PE queue never stalls waiting
        # for the ACT scale-copy — PE stays continuously busy and ramps to
        # its full 2.4 GHz p-state instead of oscillating at half speed.
        #
        # The group schedule is tapered: big groups for the bulk (DMA
        # efficiency), small final group so the compute tail after the last
        # input byte lands is one tile, not a whole group.
        GROUP = group  # pixels per input DMA (4096 -> 4 MB at fp8)
        schedule = []
        left = PIX
        while left > 0:
            gn = min(GROUP, left)
            schedule.append(gn)
            left -= gn
        if schedule[-1] > 2 * TILE_N and len(schedule) >= 1:
            last = schedule.pop()
            schedule += [last - TILE_N, TILE_N]
        pending = None
        dma_q = []
        for _rep in range(repeat):
            g0 = 0
            for gn in schedule:
                xbuf = xpool.tile([128, KCH, gn], xdt, tag="xbuf")
                nc.sync.dma_start(
                    out=xbuf[:, :, :], in_=xs_v[:, :, g0 : g0 + gn]
                )
                off = 0
                while off < gn:
                    m = min(TILE_N, gn - off)
                    nxt = emit_mms(xbuf, off, g0 + off, m)
                    if pending is not None:
                        st = emit_rest(pending)
                        if st is not None:
                            dma_q.append(st)
                        if len(dma_q) > odefer:
                            emit_dma(dma_q.pop(0))
                    pending = nxt
                    off += m
                g0 += gn
        if pending is not None:
            st = emit_rest(pending)
            if st is not None:
                dma_q.append(st)
        for staged in dma_q:
            emit_dma(staged)

    nc.compile()
    return nc


def _get_program(repeat=1, group=4096, xbufs=3, mode="fp8hi",
                 trmode="pemm", oring="scalar", odefer=1, stages=7):
    key = (repeat, group, xbufs, mode, trmode, oring, odefer, stages)
    if key not in _compiled:
        _compiled[key] = _build_program(
            repeat, group, xbufs, mode, trmode, oring, odefer, stages
        )
    return _compiled[key]


def _make_in_maps(x, cls_w, cls_b, box_w, box_b, dir_w, dir_b, mode="fp8hi"):
    w_all = np.concatenate(
        [np.asarray(cls_w), np.asarray(box_w), np.asarray(dir_w)], axis=0
    ).astype(np.float32)  # (72, 1024)
    bias_all = np.concatenate(
        [np.asarray(cls_b), np.asarray(box_b), np.asarray(dir_b)]
    ).astype(np.float32)  # (72,)

    if mode == "fp8dr":
        s = np.abs(w_all).max(axis=1) / WSCALE_TARGET  # (72,)
        wp = w_all / s[:, None]
        w_hi = wp.astype(E4M3)
        w_lo = (wp - w_hi.astype(np.float32)).astype(E4M3)
        # wt[p, h*KCH + k, o] = w_{hi,lo}[o, k*128 + p]
        whl = np.stack([w_hi, w_lo])  # (2, 72, 1024)
        wt = np.zeros((128, 2 * KCH, WPAD), dtype=E4M3)
        wt[:, :, :O] = whl.reshape(2, O, KCH, 128).transpose(3, 0, 2, 1).reshape(
            128, 2 * KCH, O
        )
        svec = s.reshape(O, 1).astype(np.float32)
        xq = np.asarray(x).astype(E4M3)
    elif mode == "fp8hi":
        # single-pass: per-channel-scaled e4m3 weights, no lo residual.
        # rel err ~1.59e-2 (vs 1.16e-2 with hi+lo), inside the 2e-2 gate.
        s = np.abs(w_all).max(axis=1) / WSCALE_TARGET  # (72,)
        w_hi = (w_all / s[:, None]).astype(E4M3)
        wt = np.zeros((128, KCH, WPAD), dtype=E4M3)
        wt[:, :, :O] = w_hi.reshape(O, KCH, 128).transpose(2, 1, 0)
        svec = s.reshape(O, 1).astype(np.float32)
        xq = np.asarray(x).astype(E4M3)
    else:
        wb = w_all.astype(ml_dtypes.bfloat16)
        wt = np.ascontiguousarray(
            wb.reshape(O, KCH, 128).transpose(2, 1, 0).reshape(128, KCH, O)
        )
        svec = np.ones((O, 1), dtype=np.float32)
        xq = np.asarray(x).astype(ml_dtypes.bfloat16)

    dmat = np.eye(O, dtype=np.float16)
    bvec = bias_all.reshape(O, 1).astype(np.float32)

    in_maps = []
    for i in range(NCORES):
        b, half = divmod(i, 2)
        xs = np.ascontiguousarray(
            xq[b, :, half * HH : (half + 1) * HH, :]
        ).reshape(C, PIX)
        in_maps.append(
            {"xs": xs, "wt": wt, "svec": svec, "bvec": bvec, "dmat": dmat}
        )
    return in_maps


def _chunks(group=4096):
    """(pix0, n) tile chunks in device-emission order, matching
    _build_program's tapered group schedule."""
    schedule = []
    left = PIX
    while left > 0:
        gn = min(group, left)
        schedule.append(gn)
        left -= gn
    if schedule[-1] > 2 * TILE_N:
        last = schedule.pop()
        schedule += [last - TILE_N, TILE_N]
    out, g0 = [], 0
    for gn in schedule:
        off = 0
        while off < gn:
            m = min(TILE_N, gn - off)
            out.append((g0 + off, m))
            off += m
        g0 += gn
    return out


def _gather(results, group=4096):
    out = np.empty((B, H, W, O), dtype=np.float32)
    for i in range(NCORES):
        b, half = divmod(i, 2)
        dev = results[i]["out"][:, :O].astype(np.float32)  # strip pad cols
        flat = np.empty((PIX, O), dtype=np.float32)
        for pix0, n in _chunks(group):
            if n % 128 == 0:
                nj = n // 128
                # interleaved tile: dev_pixel = pix0 + p*nj + j
                flat[pix0 : pix0 + n] = (
                    dev[pix0 : pix0 + n]
                    .reshape(128, nj, O)
                    .transpose(1, 0, 2)
                    .reshape(n, O)
                )
            else:
                flat[pix0 : pix0 + n] = dev[pix0 : pix0 + n]
        out[b, half * HH : (half + 1) * HH] = flat.reshape(HH, W, O)
    return out


def kernel(x, cls_w, cls_b, box_w, box_b, dir_w, dir_b):
    nc = _get_program()
    in_maps = _make_in_maps(x, cls_w, cls_b, box_w, box_b, dir_w, dir_b)
    res = run_bass_kernel_spmd(nc, in_maps, list(range(NCORES)))
    return _gather(res.results)



# revision 18
# speedup vs baseline: 1.5953x; 1.3232x over previous
"""DetectHead (three 1x1-conv heads fused) on 8 Trainium2 NeuronCores.

Math: out[b,h,w,:] = concat(cls, box, dir) = W_all @ x[b,:,h,w] + bias_all
with W_all = concat(cls_w, box_w, dir_w) in R^{72x1024}.

Sharding: 8 shards = (batch, H-half). Each core processes a contiguous
(1024, 100*176=17600) slice of x and produces (17600, 72) of the
channels-last output.

The kernel is HBM-read-bound, so the main lever is input bytes. x is
quantized host-side to fp8 e4m3 (relative rounding err 2^-4; the matmul
contracts 1024 of them so the output rel err lands at ~1.1e-2, inside the
2e-2 gate).  Weights are also e4m3 — required for the PE's DoubleRow perf
mode (2 K-rows/cycle, both operands must be fp8e4/e5) — but split per
output channel o into  w = s_o * (hi + lo)  with hi = e4m3(w/s_o),
lo = e4m3(w/s_o - hi), s_o = max|w_o|/240.  The per-channel scale keeps
box_w (~1e-3) clear of e4m3's 2^-9 subnormal floor and the hi+lo pair
kills the weight quantization error (~1e-3 residual).

Per-tile pipeline (n=512 pixels):
  - 8 accumulating DoubleRow matmuls (4 k-pair chunks x {hi,lo},
    0.5 cycles/row; weight k-pitch padded to 80 for the ldweights
    step%16==0 ISA rule) -> PSUM [72,512] fp32
  - ACT engine activation(Copy, scale=s_o) applies the dequant scale
    during the PSUM->SBUF copy, writing fp16 rows 0..71 of a [73,512]
    staging tile whose row 72 is a persistent 1.0 (primed once)
  - 4 PE transposes against D[73,72] = [I; bias_row] — the ones row adds
    the bias for free — writing fp16 [128,72] tiles to PSUM
  - DVE copies the fp16 [128, nj*72] PSUM tile to SBUF (~150 ns),
    one DMA on the ACT ring writes the tile with dev_pixel = p*4 + j
    (576 B contiguous per partition -> full DMA efficiency); the host
    de-interleaves when gathering.

Engine budget per core: DMA ~57 us (17.6 MB fp8 in + 2.5 MB fp16 out at
360 GB/s) is the roofline; PE ~34-50 us (p-state dependent), ACT ~15 us,
DVE ~5 us all hide under it.
"""

import numpy as np
from contextlib import ExitStack

import ml_dtypes

import concourse.bass as bass
import concourse.tile as tile
from concourse import bacc, mybir
from concourse.bass_utils import run_bass_kernel_spmd

B, C, H, W = 4, 1024, 200, 176
HH = H // 2            # 100 rows of H per shard
PIX = HH * W           # 17600 pixels per shard
NCORES = 8
KCH = C // 128         # 8 channel chunks
O = 72                 # 18 cls + 42 box + 12 dir output channels
TILE_N = 512
FULL_TILES = PIX // TILE_N          # 34
TAIL = PIX - FULL_TILES * TILE_N    # 192

F32 = mybir.dt.float32
F16 = mybir.dt.float16
BF16 = mybir.dt.bfloat16
F8E4 = mybir.dt.float8e4
WPAD = 80  # ktile stride for fp8 weights: DoubleRow ldweights needs step%16==0

E4M3 = ml_dtypes.float8_e4m3
WSCALE_TARGET = 240.0  # normalize max|w_o| to this inside e4m3's range

_compiled = {}


def _build_program(repeat=1, group=4096, xbufs=3, mode="fp8hi",
                   trmode="xbar", oring="scalar", odefer=2, stages=7):
    nc = bacc.Bacc(
        "TRN2", target_bir_lowering=False, debug=False, num_devices=NCORES
    )
    if mode == "fp8dr":
        xdt, wdt, n_wk, wpitch = F8E4, F8E4, 2 * KCH, WPAD
    elif mode == "fp8hi":
        xdt, wdt, n_wk, wpitch = F8E4, F8E4, KCH, WPAD
    elif mode == "bf16":
        xdt, wdt, n_wk, wpitch = BF16, BF16, KCH, O
    else:
        raise ValueError(mode)

    xs = nc.dram_tensor("xs", [C, PIX], xdt, kind="ExternalInput").ap()
    wt = nc.dram_tensor("wt", [128, n_wk, wpitch], wdt, kind="ExternalInput").ap()
    svec = nc.dram_tensor("svec", [O, 1], F32, kind="ExternalInput").ap()
    bvec = nc.dram_tensor("bvec", [O, 1], F32, kind="ExternalInput").ap()
    dmat = nc.dram_tensor("dmat", [O, O], F16, kind="ExternalInput").ap()
    OP = 80 if trmode == "xbar" else O  # xbar: pad to %16 partitions
    out = nc.dram_tensor("out", [PIX, OP], F16, kind="ExternalOutput").ap()

    # [c, pix] viewed as [p, k, pix] with c = k*128 + p
    xs_v = xs.rearrange("(k p) n -> p k n", k=KCH)

    with tile.TileContext(nc) as tc, ExitStack() as ctx:
        cpool = ctx.enter_context(tc.tile_pool(name="consts", bufs=1))
        xpool = ctx.enter_context(tc.tile_pool(name="xin", bufs=xbufs))
        opool = ctx.enter_context(
            tc.tile_pool(name="outsb", bufs=3 + odefer)
        )
        mpool = ctx.enter_context(tc.tile_pool(name="pmm", bufs=2, space="PSUM"))
        tpool = ctx.enter_context(tc.tile_pool(name="ptr", bufs=2, space="PSUM"))
        dma_eng = {"scalar": nc.scalar, "sync": nc.sync,
                   "gpsimd": nc.gpsimd}[oring]

        w_sb = cpool.tile([128, n_wk, wpitch], wdt)
        nc.sync.dma_start(out=w_sb[:, :, :], in_=wt[:, :, :])
        s_sb = cpool.tile([O, 1], F32)
        nc.sync.dma_start(out=s_sb[:, :], in_=svec[:, :])
        b_sb = cpool.tile([O, 1], F32)
        nc.sync.dma_start(out=b_sb[:, :], in_=bvec[:, :])
        d_sb = cpool.tile([O, O], F16)
        nc.sync.dma_start(out=d_sb[:, :], in_=dmat[:, :])
        spool = ctx.enter_context(tc.tile_pool(name="stage", bufs=3))

        def emit_mms(xbuf, off, pix0, n):
            # accumulating matmuls for one n<=512 pixel tile -> PSUM [72, n]
            pmm = mpool.tile([O, n], F32, tag="pmm")
            if mode == "fp8dr":
                # 4 k-pair chunks x {hi, lo} accumulating DoubleRow matmuls
                for h in range(2):
                    for j in range(KCH // 2):
                        nc.tensor.matmul(
                            pmm[:, :],
                            w_sb[:, h * KCH + 2 * j : h * KCH + 2 * j + 2, :O],
                            xbuf[:, 2 * j : 2 * j + 2, off : off + n],
                            start=(h == 0 and j == 0),
                            stop=(h == 1 and j == KCH // 2 - 1),
                            perf_mode=mybir.MatmulPerfMode.DoubleRow,
                        )
            elif mode == "fp8hi":
                # 4 k-pair chunks, hi-precision weights only (single pass)
                for j in range(KCH // 2):
                    nc.tensor.matmul(
                        pmm[:, :],
                        w_sb[:, 2 * j : 2 * j + 2, :O],
                        xbuf[:, 2 * j : 2 * j + 2, off : off + n],
                        start=(j == 0),
                        stop=(j == KCH // 2 - 1),
                        perf_mode=mybir.MatmulPerfMode.DoubleRow,
                    )
            else:
                for k in range(KCH):
                    nc.tensor.matmul(
                        pmm[:, :],
                        w_sb[:, k, :O],
                        xbuf[:, k, off : off + n],
                        start=(k == 0),
                        stop=(k == KCH - 1),
                    )
            return pmm, pix0, n

        def emit_rest(pending):
            # scale-copy, transposes, SBUF stage, output DMA for a tile
            pmm, pix0, n = pending
            if not stages & 1:
                return None
            njs = [128] * (n // 128)
            if n % 128:
                njs.append(n % 128)
            nj = len(njs)

            # ACT: PSUM -> SBUF fp16, dequant scale and bias fused
            # (pre-transpose the output channel o is the partition dim, so
            # both are per-partition [72,1] vectors)
            s1 = spool.tile([OP if trmode == "xbar" else O, TILE_N], F16,
                            tag="s1")
            nc.scalar.activation(
                s1[:O, :n], pmm[:, :],
                mybir.ActivationFunctionType.Identity,
                bias=b_sb[:, :],
                scale=s_sb[:, :],
            )

            if not stages & 2:
                return None
            if trmode == "xbar":
                # X-bar DMA transpose, SBUF->SBUF on the HWDGE scalar ring:
                # [80,128] -> [128,80] per block; no PE/DVE involvement and
                # SBUF<->SBUF traffic does not eat the HBM budget.  Rows
                # 72..79 of s1 are garbage; they ship as cols 72..80 and the
                # host strips them.  The 64-px tail block (free dim %128 != 0)
                # falls back to the PE path below.
                ot = opool.tile([128, nj * OP], F16, tag="ot")
                pemm_js = []
                for j, pj in enumerate(njs):
                    if pj % 128 == 0:
                        nc.scalar.dma_start_transpose(
                            ot[:, j * OP : (j + 1) * OP],
                            s1[:, j * 128 : j * 128 + pj],
                        )
                    else:
                        pemm_js.append((j, pj))
                if pemm_js:
                    pt = tpool.tile([128, len(pemm_js) * O], F32, tag="pt")
                    for i, (j, pj) in enumerate(pemm_js):
                        nc.tensor.matmul(
                            pt[:pj, i * O : (i + 1) * O],
                            s1[:O, j * 128 : j * 128 + pj],
                            d_sb[:, :],
                            start=True,
                            stop=True,
                        )
                        nc.vector.tensor_copy(
                            ot[:pj, j * OP : j * OP + O],
                            pt[:pj, i * O : (i + 1) * O],
                        )
                return ot, pix0, n, njs, nj
            # transpose via REGULAR matmul against an identity rhs:
            # out[pj,72] = s1_j.T @ I.  Regular matmuls warm-clock (2.4 GHz)
            # and FWL-load the fp16 stationary operand; transpose-mode
            # (nc.tensor.transpose) stays cold at 1.2 GHz.  Output is fp32
            # in PSUM; the DVE copy casts to fp16 on the way to SBUF.
            if trmode == "pemm":
                pt = tpool.tile([128, nj * O], F32, tag="pt")
                for j, pj in enumerate(njs):
                    nc.tensor.matmul(
                        pt[:pj, j * O : (j + 1) * O],
                        s1[:, j * 128 : j * 128 + pj],
                        d_sb[:, :],
                        start=True,
                        stop=True,
                    )
            else:
                pt = tpool.tile([128, nj * O], F16, tag="pt")
                for j, pj in enumerate(njs):
                    nc.tensor.transpose(
                        pt[:pj, j * O : (j + 1) * O],
                        s1[:, j * 128 : j * 128 + pj],
                        d_sb[:, :],
                    )

            ot = opool.tile([128, nj * O], F16, tag="ot")
            nc.vector.tensor_copy(ot[: max(njs), : nj * O], pt[: max(njs), : nj * O])
            return ot, pix0, n, njs, nj

        def emit_dma(staged):
            # output DMA, deferred `odefer` tiles so the dma_start never
            # waits at its queue head for this tile's DVE copy (which would
            # block the next ACT behind it on a strict-FIFO engine queue)
            ot, pix0, n, njs, nj = staged
            if not stages & 4:
                return
            W_ = OP if trmode == "xbar" else O
            if n % 128 == 0:
                # dev layout: dev_pixel = pix0 + p*nj + j  (>=576 B contiguous
                # per partition -> no sub-512B DMA penalty); host unpermutes.
                dma_eng.dma_start(
                    out=out[pix0 : pix0 + n, :].rearrange(
                        "(p j) o -> p j o", p=128
                    ),
                    in_=ot[:, : nj * W_].rearrange("p (j o) -> p j o", j=nj),
                )
            else:
                for j, pj in enumerate(njs):
                    dma_eng.dma_start(
                        out=out[pix0 + j * 128 : pix0 + j * 128 + pj, :],
                        in_=ot[:pj, j * W_ : j * W_ + W_],
                    )

        # Software-pipelined emission: tile t+1's matmuls are issued BEFORE
        # tile t's transposes so the in-order PE queue never stalls waiting
        # for the ACT scale-copy — PE stays continuously busy and ramps to
        # its full 2.4 GHz p-state instead of oscillating at half speed.
        #
        # The group schedule is tapered: big groups for the bulk (DMA
        # efficiency), small final group so the compute tail after the last
        # input byte lands is one tile, not a whole group.
        GROUP = group  # pixels per input DMA (4096 -> 4 MB at fp8)
        schedule = []
        left = PIX
        while left > 0:
            gn = min(GROUP, left)
            schedule.append(gn)
            left -= gn
        if schedule[-1] > 2 * TILE_N and len(schedule) >= 1:
            last = schedule.pop()
            schedule += [last - TILE_N, TILE_N]
        pending = None
        dma_q = []
        for _rep in range(repeat):
            g0 = 0
            for gn in schedule:
                xbuf = xpool.tile([128, KCH, gn], xdt, tag="xbuf")
                nc.sync.dma_start(
                    out=xbuf[:, :, :], in_=xs_v[:, :, g0 : g0 + gn]
                )
                off = 0
                while off < gn:
                    m = min(TILE_N, gn - off)
                    nxt = emit_mms(xbuf, off, g0 + off, m)
                    if pending is not None:
                        st = emit_rest(pending)
                        if st is not None:
                            dma_q.append(st)
                        if len(dma_q) > odefer:
                            emit_dma(dma_q.pop(0))
                    pending = nxt
                    off += m
                g0 += gn
        if pending is not None:
            st = emit_rest(pending)
            if st is not None:
                dma_q.append(st)
        for staged in dma_q:
            emit_dma(staged)

    nc.compile()
    return nc


def _get_program(repeat=1, group=4096, xbufs=3, mode="fp8hi",
                 trmode="xbar", oring="scalar", odefer=2, stages=7):
    key = (repeat, group, xbufs, mode, trmode, oring, odefer, stages)
    if key not in _compiled:
        _compiled[key] = _build_program(
            repeat, group, xbufs, mode, trmode, oring, odefer, stages
        )
    return _compiled[key]


def _make_in_maps(x, cls_w, cls_b, box_w, box_b, dir_w, dir_b, mode="fp8hi"):
    w_all = np.concatenate(
        [np.asarray(cls_w), np.asarray(box_w), np.asarray(dir_w)], axis=0
    ).astype(np.float32)  # (72, 1024)
    bias_all = np.concatenate(
        [np.asarray(cls_b), np.asarray(box_b), np.asarray(dir_b)]
    ).astype(np.float32)  # (72,)

    if mode == "fp8dr":
        s = np.abs(w_all).max(axis=1) / WSCALE_TARGET  # (72,)
        wp = w_all / s[:, None]
        w_hi = wp.astype(E4M3)
        w_lo = (wp - w_hi.astype(np.float32)).astype(E4M3)
        # wt[p, h*KCH + k, o] = w_{hi,lo}[o, k*128 + p]
        whl = np.stack([w_hi, w_lo])  # (2, 72, 1024)
        wt = np.zeros((128, 2 * KCH, WPAD), dtype=E4M3)
        wt[:, :, :O] = whl.reshape(2, O, KCH, 128).transpose(3, 0, 2, 1).reshape(
            128, 2 * KCH, O
        )
        svec = s.reshape(O, 1).astype(np.float32)
        xq = np.asarray(x).astype(E4M3)
    elif mode == "fp8hi":
        # single-pass: per-channel-scaled e4m3 weights, no lo residual.
        # rel err ~1.59e-2 (vs 1.16e-2 with hi+lo), inside the 2e-2 gate.
        s = np.abs(w_all).max(axis=1) / WSCALE_TARGET  # (72,)
        w_hi = (w_all / s[:, None]).astype(E4M3)
        wt = np.zeros((128, KCH, WPAD), dtype=E4M3)
        wt[:, :, :O] = w_hi.reshape(O, KCH, 128).transpose(2, 1, 0)
        svec = s.reshape(O, 1).astype(np.float32)
        xq = np.asarray(x).astype(E4M3)
    else:
        wb = w_all.astype(ml_dtypes.bfloat16)
        wt = np.ascontiguousarray(
            wb.reshape(O, KCH, 128).transpose(2, 1, 0).reshape(128, KCH, O)
        )
        svec = np.ones((O, 1), dtype=np.float32)
        xq = np.asarray(x).astype(ml_dtypes.bfloat16)

    dmat = np.eye(O, dtype=np.float16)
    bvec = bias_all.reshape(O, 1).astype(np.float32)

    in_maps = []
    for i in range(NCORES):
        b, half = divmod(i, 2)
        xs = np.ascontiguousarray(
            xq[b, :, half * HH : (half + 1) * HH, :]
        ).reshape(C, PIX)
        in_maps.append(
            {"xs": xs, "wt": wt, "svec": svec, "bvec": bvec, "dmat": dmat}
        )
    return in_maps


def _chunks(group=4096):
    """(pix0, n) tile chunks in device-emission order, matching
    _build_program's tapered group schedule."""
    schedule = []
    left = PIX
    while left > 0:
        gn = min(group, left)
        schedule.append(gn)
        left -= gn
    if schedule[-1] > 2 * TILE_N:
        last = schedule.pop()
        schedule += [last - TILE_N, TILE_N]
    out, g0 = [], 0
    for gn in schedule:
        off = 0
        while off < gn:
            m = min(TILE_N, gn - off)
            out.append((g0 + off, m))
            off += m
        g0 += gn
    return out


def _gather(results, group=4096):
    out = np.empty((B, H, W, O), dtype=np.float32)
    for i in range(NCORES):
        b, half = divmod(i, 2)
        dev = results[i]["out"][:, :O].astype(np.float32)  # strip pad cols
        flat = np.empty((PIX, O), dtype=np.float32)
        for pix0, n in _chunks(group):
            if n % 128 == 0:
                nj = n // 128
                # interleaved tile: dev_pixel = pix0 + p*nj + j
                flat[pix0 : pix0 + n] = (
                    dev[pix0 : pix0 + n]
                    .reshape(128, nj, O)
                    .transpose(1, 0, 2)
                    .reshape(n, O)
                )
            else:
                flat[pix0 : pix0 + n] = dev[pix0 : pix0 + n]
        out[b, half * HH : (half + 1) * HH] = flat.reshape(HH, W, O)
    return out


def kernel(x, cls_w, cls_b, box_w, box_b, dir_w, dir_b):
    nc = _get_program()
    in_maps = _make_in_maps(x, cls_w, cls_b, box_w, box_b, dir_w, dir_b)
    res = run_bass_kernel_spmd(nc, in_maps, list(range(NCORES)))
    return _gather(res.results)

